# revision 3
# baseline (speedup 1.0000x reference)
"""Binary vector-quantizer kernel for Trainium2 (8 NeuronCores, data-parallel).

The reference codebook enumerates ALL 2^12 binary codes, so the nearest-code
search is separable per coordinate: bit_j = (x_j > 0.5), the code index is the
12-bit integer (MSB-first), quantized == bits, and the commitment loss is
mean((bits - x)^2).  Verified exact against the matmul+argmin reference on the
actual inputs (the closest element to the 0.5 boundary is 2.4e-7 away and
still agrees).

Per core shard (8192 rows = 128 partitions x 64 rows, two 32-row chunks):
  SP sequencer:  chunk input DMAs, packed aux output DMA
  ACT sequencer: quantized (u8) output DMA
  DVE:           is_gt threshold, 4-op index tree, squared-diff accumulate
  Pool (GPSIMD): d = bits - x  (runs in parallel with the DVE tree)
Raw bacc (no TileContext) with hand-placed semaphores: Tile's preamble/tail
barriers cost ~4us on a ~9us kernel.  DVE ops are interleaved across the two
chunks so same-engine RAW semaphores propagate during the intervening op.

Outputs per core: q_out u8 [8192,12] (exact 0/1), aux_out f32 [128,66] =
64 index columns (exact ints <= 4095) + 2 per-partition loss partial sums.
The host converts dtypes, concatenates shards, and finishes the scalar mean.
"""

import numpy as np

import concourse.bacc as bacc
from concourse import mybir
from concourse import bass_utils

N_FULL = 65536
L = 12
N_CORES = 8
N_SHARD = N_FULL // N_CORES     # 8192 rows per core
P = 128                         # SBUF partitions
RPP = N_SHARD // P              # 64 rows per partition
HALF = RPP // 2                 # 32-row chunks
AUX_W = RPP + 2                 # 64 idx cols + 2 loss cols

F32, U8 = mybir.dt.float32, mybir.dt.uint8
Alu = mybir.AluOpType


def build_nc():
    nc = bacc.Bacc(
        "TRN2",
        target_bir_lowering=False,
        debug=False,
        enable_asserts=False,
        num_devices=N_CORES,
    )

    x_dram = nc.dram_tensor("x_in", [N_SHARD, L], F32, kind="ExternalInput")
    q_dram = nc.dram_tensor("q_out", [N_SHARD, L], U8, kind="ExternalOutput")
    aux_dram = nc.dram_tensor("aux_out", [P, AUX_W], F32, kind="ExternalOutput")

    x_v = x_dram.ap().rearrange("(p n) l -> p n l", p=P)    # [128, 64, 12]
    q_v = q_dram.ap().rearrange("(p n) l -> p n l", p=P)

    x_t = nc.alloc_sbuf_tensor("x_t", [P, RPP, L], F32).ap()
    bits = nc.alloc_sbuf_tensor("bits", [P, RPP, L], U8).ap()
    d_t = nc.alloc_sbuf_tensor("d_t", [P, RPP, L], F32).ap()
    junk = nc.alloc_sbuf_tensor("junk", [P, RPP, L], F32).ap()
    s1 = nc.alloc_sbuf_tensor("s1", [P, RPP, 6], F32).ap()
    s2 = nc.alloc_sbuf_tensor("s2", [P, RPP, 3], F32).ap()
    t4 = nc.alloc_sbuf_tensor("t4", [P, RPP], F32).ap()
    aux = nc.alloc_sbuf_tensor("aux", [P, AUX_W], F32).ap()

    in_sems = [nc.alloc_semaphore("in_sem0"), nc.alloc_semaphore("in_sem1")]
    p_sem = nc.alloc_semaphore("p_sem")
    v_sem = nc.alloc_semaphore("v_sem")
    qd_sem = nc.alloc_semaphore("qd_sem")
    auxd_sem = nc.alloc_semaphore("auxd_sem")

    A, B = slice(0, HALF), slice(HALF, RPP)

    # SP: chunk input DMAs.  HWDGE completions are not FIFO-ordered across
    # dma_starts, hence one semaphore per chunk.
    nc.sync.dma_start(out=x_t[:, A, :], in_=x_v[:, A, :]).then_inc(in_sems[0], 16)
    nc.sync.dma_start(out=x_t[:, B, :], in_=x_v[:, B, :]).then_inc(in_sems[1], 16)

    def gt(R, in_sem):
        return nc.vector.tensor_scalar(
            out=bits[:, R, :], in0=x_t[:, R, :], scalar1=0.5, scalar2=None,
            op0=Alu.is_gt)._wait_ge(in_sem, 16)

    def tree(stage, R, wait_v):
        if stage == 0:
            ins = nc.vector.scalar_tensor_tensor(
                out=s1[:, R, :], in0=bits[:, R, 0::2], scalar=2.0,
                in1=bits[:, R, 1::2], op0=Alu.mult, op1=Alu.add)
        elif stage == 1:
            ins = nc.vector.scalar_tensor_tensor(
                out=s2[:, R, :], in0=s1[:, R, 0::2], scalar=4.0,
                in1=s1[:, R, 1::2], op0=Alu.mult, op1=Alu.add)
        elif stage == 2:
            ins = nc.vector.scalar_tensor_tensor(
                out=t4[:, R], in0=s2[:, R, 0], scalar=16.0,
                in1=s2[:, R, 1], op0=Alu.mult, op1=Alu.add)
        else:
            ins = nc.vector.scalar_tensor_tensor(
                out=aux[:, R], in0=t4[:, R], scalar=16.0,
                in1=s2[:, R, 2], op0=Alu.mult, op1=Alu.add)
        return ins._wait_ge(v_sem, wait_v)

    def d2(col, R, wait_p):
        return nc.vector.scalar_tensor_tensor(
            out=junk[:, R, :], in0=d_t[:, R, :], scalar=1.0,
            in1=d_t[:, R, :], op0=Alu.mult, op1=Alu.mult,
            accum_out=aux[:, RPP + col : RPP + col + 1])._wait_ge(p_sem, wait_p)

    # DVE schedule (v_sem counts in comments):
    gt(A, in_sems[0]).then_inc(v_sem, 1)   # v1
    tree(0, A, 1).then_inc(v_sem, 1)       # v2
    tree(1, A, 2).then_inc(v_sem, 1)       # v3
    gt(B, in_sems[1]).then_inc(v_sem, 1)   # v4
    tree(2, A, 3).then_inc(v_sem, 1)       # v5
    tree(0, B, 4).then_inc(v_sem, 1)       # v6
    tree(3, A, 5).then_inc(v_sem, 1)       # v7
    tree(1, B, 6).then_inc(v_sem, 1)       # v8
    d2(0, A, 1).then_inc(v_sem, 1)         # v9
    tree(2, B, 8).then_inc(v_sem, 1)       # v10
    d2(1, B, 2).then_inc(v_sem, 1)         # v11
    tree(3, B, 10).then_inc(v_sem, 1)      # v12

    # Pool: d = bits - x, per chunk, gated on the matching is_gt.
    nc.gpsimd.tensor_tensor(
        out=d_t[:, A, :], in0=bits[:, A, :], in1=x_t[:, A, :],
        op=Alu.subtract)._wait_ge(v_sem, 1).then_inc(p_sem, 1)
    nc.gpsimd.tensor_tensor(
        out=d_t[:, B, :], in0=bits[:, B, :], in1=x_t[:, B, :],
        op=Alu.subtract)._wait_ge(v_sem, 4).then_inc(p_sem, 1)

    # ACT: quantized output once both is_gt ops retired (v_sem >= 4).
    nc.scalar.dma_start(out=q_v, in_=bits)._wait_ge(v_sem, 4).then_inc(qd_sem, 16)
    # SP: packed aux output after the last DVE op (engine retires in order).
    nc.sync.dma_start(out=aux_dram.ap(), in_=aux)._wait_ge(v_sem, 12).then_inc(
        auxd_sem, 16)
    nc.sync.wait_ge(auxd_sem, 16)
    nc.scalar.wait_ge(qd_sem, 16)

    nc.compile()
    return nc


_NC_CACHE = []


def _get_nc():
    if not _NC_CACHE:
        _NC_CACHE.append(build_nc())
    return _NC_CACHE[0]


def kernel(inputs, codebook=None):
    x = np.ascontiguousarray(np.asarray(inputs, dtype=np.float32))
    assert x.shape == (N_FULL, L), x.shape
    nc = _get_nc()

    in_maps = [
        {"x_in": x[i * N_SHARD : (i + 1) * N_SHARD]} for i in range(N_CORES)
    ]
    res = bass_utils.run_bass_kernel_spmd(nc, in_maps, core_ids=list(range(N_CORES)))
    rs = res.results

    q = np.concatenate([r["q_out"] for r in rs], axis=0).astype(np.float32)
    idx = np.concatenate(
        [r["aux_out"][:, :RPP].reshape(-1) for r in rs], axis=0
    ).astype(np.int32)
    total = np.sum([r["aux_out"][:, RPP:].astype(np.float64).sum() for r in rs])
    loss = np.float32(total / (N_FULL * L))
    return q, loss, idx
